# revision 26
# baseline (speedup 1.0000x reference)
"""Trainium2 Bass kernel for nn_BaselineAttention_25984552141259.

Problem: QKV [3, B=2, H=8, N=4096, d=64] fp32 ->
         out[b,h,n,:] = softmax(Q[b,h] @ K[b,h].T) @ V[b,h]

Sharding: B*H = 16 heads -> 2 heads per core on 8 NeuronCores.

v3 design. v1 was ACT-bound (~285us: one 128-lane 1.2 GHz activation
engine exp-ing N^2 scores) with the PE at 89%; v3 attacks both:

  1. Row-tiled S^T matmuls in fp16. d=64 contraction only fills half
     the PE array, so two m-blocks run CONCURRENTLY as K=64 tiles in
     row groups (0,0) and (64,0): K^T host-packed with even m-blocks on
     partitions 0-63 / odd on 64-127, Q^T duplicated into both halves.
     fp16 (not fp32r) because fp32r matmuls fuse their weight load and
     serialize it with the stream (measured 427ns/pair); fp16 gets
     standalone LDWEIGHTS + fast-weight-load, so pairs run
     stream-bound at ~213ns. fp16's 10 mantissa bits keep the score
     error ~4x below bf16's.
  2. exp split across engines, output in bf16. Per n-chunk the 16
     m-block pairs are exp'd 9 on ACT (table exp -> bf16) and 7 on DVE
     via Schraudolph's bit trick in bf16-bit space:
     i16 = s*(2^7/ln2) + (127*2^7 - C - 25*2^7/ln2), the int16
     reinterpreted as bf16 ~= exp(s-25) with +-3.3% sawtooth error.
     One fused tensor_scalar (mult, add, int16 convert-on-write) per
     tile, free bitcast view for the PE. Softmax renormalizes by the
     matmul'd row sums, so the approximation only redistributes
     weight; simulated end-to-end rel err 1.5e-2 vs the 2e-2 gate.
  3. O^T[d', n] = sum_m V'[m, d'] * P^T[m, n] accumulated over all 32
     m-blocks per chunk, V'/P in bf16 (V' = [V | ones] so row 64 is
     the softmax denominator), fp32 PSUM accumulation.
  4. Normalization: single-DVE-op approx reciprocal (~18 bits) on the
     denominator row, hopped through SBUF first (custom-DVE ops read
     garbage from PSUM, measured). Broadcast of 1/denom across
     partitions via DRAM-bounce stride-0 DMA so it never enters the PE
     queue.
"""
import numpy as np
from contextlib import ExitStack

import concourse.bass as bass
import concourse.tile as tile
from concourse import bacc, mybir
from concourse.bass_utils import run_bass_kernel_spmd

N_CORES = 8
B, H, N, D = 2, 8, 4096, 64
HEADS = B * H
HPC = HEADS // N_CORES          # heads per core = 2
NCHUNK = 512                    # n-tile (matmul moving free dim)
NCH = N // NCHUNK               # 8 n-chunks per head
MB = N // 128                   # 32 m-blocks of 128 keys
NPAIR = MB // 2                 # 16 row-tiled m-block pairs
PPIECE = NPAIR // 4             # pairs per K^T load piece
EXP_BIAS = -25.0

# Schraudolph exp in bf16-bit space: exp(x) ~= bitcast_bf16(
# i16(A16*x + 127*2^7 - C16)); C16 minimizes max relative error
# (+-3.3%); bias -25 folded into B16.
SCHRAU_A = 2.0 ** 7 / np.log(2.0)             # 184.6650...
SCHRAU_B = float(np.float32(127 * 2 ** 7 - 5.5 + EXP_BIAS * SCHRAU_A))
# DVE handles these m-block pairs each chunk (7/16 ~= throughput balance
# vs ACT; pattern's end-to-end rel err simulated at ~1.5e-2 max).
DVE_PAIRS = frozenset((0, 2, 4, 7, 9, 11, 14))

F32 = mybir.dt.float32
F32R = mybir.dt.float32r
F16 = mybir.dt.float16
BF16 = mybir.dt.bfloat16
I16 = mybir.dt.int16

_CACHE = {}


def _build():
    nc = bacc.Bacc("TRN2", target_bir_lowering=False, debug=False,
                   num_devices=N_CORES)
    # qt: Q^T duplicated into both 64-partition halves (row-tile moving
    # operand). kt: even m-blocks on partitions 0-63, odd on 64-127,
    # pair-major in the free dim. Both host-packed fp16.
    qt_d = nc.dram_tensor("qt", [HPC, 128, N], F16, kind="ExternalInput").ap()
    kt_d = nc.dram_tensor("kt", [HPC, 128, NPAIR * 128], F16,
                          kind="ExternalInput").ap()
    v_d = nc.dram_tensor("v", [HPC, N, D], BF16, kind="ExternalInput").ap()
    ot_d = nc.dram_tensor("ot", [HPC, D, N], F32, kind="ExternalOutput").ap()

    with tile.TileContext(nc) as tc, ExitStack() as ctx:
        const = ctx.enter_context(tc.tile_pool(name="const", bufs=1))
        qk = ctx.enter_context(tc.tile_pool(name="qk", bufs=2))
        vpool = ctx.enter_context(tc.tile_pool(name="vpool", bufs=2))
        pexp = ctx.enter_context(tc.tile_pool(name="pexp", bufs=5))
        opool = ctx.enter_context(tc.tile_pool(name="opool", bufs=3))
        rpool = ctx.enter_context(tc.tile_pool(name="rpool", bufs=2))
        s_ps = ctx.enter_context(tc.tile_pool(name="s_ps", bufs=3, space="PSUM"))
        ot_ps = ctx.enter_context(tc.tile_pool(name="ot_ps", bufs=2, space="PSUM"))
        rdram = ctx.enter_context(tc.tile_pool(name="rdram", bufs=2, space="DRAM"))

        bias_t = const.tile([128, 1], F32)
        nc.vector.memset(bias_t[:], EXP_BIAS)
        vone_f = const.tile([128, MB], BF16)
        nc.vector.memset(vone_f[:], 1.0)
        one_f = const.tile([1, 1], F32)
        nc.vector.memset(one_f[:], 1.0)
        ones_r = const.tile([1, D], F32R)
        nc.vector.tensor_copy(ones_r[:], one_f[:].to_broadcast((1, D)))

        # PE warmup: the HAM clock gate needs ~3.4us of sustained matmul
        # activity to lift the PE from 1.2 to 2.4 GHz, and the first ~12us
        # of the kernel are DMA-wait anyway. Rotate dummy matmuls through
        # the s_ps ring (3 bufs -> pipelined, done by ~9us, so the ring is
        # free again before the first real S-pair lands).
        warm_src = const.tile([128, NCHUNK], F16)
        nc.vector.memset(warm_src[:], 0.25)
        for w in range(18):
            wp = s_ps.tile([128, NCHUNK], F32, tag="s", name=f"warm{w}")
            nc.tensor.matmul(wp[:], warm_src[0:64, 0:128],
                             warm_src[0:64, :], start=True, stop=True,
                             tile_position=(0, 0))

        kt_all, qt_all, v_all = [], [], []
        for h in range(HPC):
            with nc.named_scope(f"load{h}"):
                # split loads so the first pairs/chunks arrive (and
                # compute starts) before the rest of the head lands
                kt_s, qt_s, v_s = [], [], []
                v_re = v_d[h].rearrange("(t p) d -> p t d", p=128)
                for i in range(4):
                    kq = qk.tile([128, PPIECE, 128], F16, tag=f"kt{i}",
                                 name=f"kt_{h}_{i}")
                    nc.sync.dma_start(
                        kq[:],
                        kt_d[h, :, bass.ts(i, PPIECE * 128)].rearrange(
                            "p (t q) -> p t q", q=128),
                    )
                    kt_s.append(kq)
                    qq = qk.tile([128, 2, NCHUNK], F16, tag=f"qt{i}",
                                 name=f"qt_{h}_{i}")
                    nc.sync.dma_start(
                        qq[:],
                        qt_d[h, :, bass.ts(i, 2 * NCHUNK)].rearrange(
                            "p (t q) -> p t q", q=NCHUNK),
                    )
                    qt_s.append(qq)
                    # V' piece [m-part, m-tile, d+1]; col 64 = 1.0 (row sums)
                    vq = vpool.tile([128, MB // 4, D + 1], BF16, tag=f"v{i}",
                                    name=f"v_{h}_{i}")
                    nc.sync.dma_start(
                        vq[:, :, 0:D],
                        v_re[:, bass.ts(i, MB // 4), :],
                    )
                    nc.vector.tensor_copy(vq[:, :, D], vone_f[:, 0:MB // 4])
                    v_s.append(vq)
                kt_all.append(kt_s)
                qt_all.append(qt_s)
                v_all.append(v_s)

        # Software-pipelined emission: the PE engine queue is strict FIFO,
        # so if O-matmuls directly follow their S-pair they stall the PE
        # for the full exp latency (~1.2us, measured as evt_wait on every
        # pair). Emitting each pair's O-matmuls LAG emissions later gives
        # the ACT/DVE exp time to land while the PE streams other pairs.
        # The two heads are interleaved as independent dependency chains
        # (fills the ~100ns/group weight-load bubbles a single chain
        # leaves), with head 1 offset by half a chunk so the chunk-end
        # normalize chains stagger and ot_ps bufs=2 never blocks.
        LAG = 6

        ot_live = {}

        def emit_o(ent):
            h, nch, pair, p_mm = ent
            if pair == 0:
                ot_live[h] = ot_ps.tile([D + 1, NCHUNK], F32, tag="ot",
                                        name=f"ot_{h}_{nch}")
            ot_t = ot_live[h]
            for j in range(2):
                m = 2 * pair + j
                nc.tensor.matmul(
                    ot_t[:],
                    v_all[h][m // (MB // 4)][:, m % (MB // 4), :],
                    p_mm[:, j, :],
                    start=(m == 0), stop=(m == MB - 1),
                )
            if pair == NPAIR - 1:
                emit_normalize(h, nch, ot_t)

        def emit_normalize(h, nch, ot_t):
            # normalize: out^T = O^T[0:64] * bcast(1 / O^T[64]).
            # single-op approx reciprocal (18 bits is plenty for a
            # softmax denominator); custom-DVE ops can't read PSUM
            # (garbage, measured) so hop the row through SBUF.
            den1 = rpool.tile([1, NCHUNK], F32, tag="den1")
            nc.scalar.copy(den1[:], ot_t[D:D + 1, :])
            rec_f = rpool.tile([1, NCHUNK], F32, tag="rec_f")
            nc.vector.reciprocal_approx_fast(rec_f[:], den1[:])
            bc_s = opool.tile([D, NCHUNK], F32, tag="bc")
            if h == HPC - 1 and nch == NCH - 1:
                # tail-only: PE K=1 broadcast matmul is ~2us faster
                # than the DRAM bounce, and at the very end the PE is
                # idle and HAM re-throttling no longer matters
                rec_r = rpool.tile([1, NCHUNK], F32R, tag="rec_r")
                nc.vector.tensor_copy(rec_r[:], rec_f[:])
                bc_t = s_ps.tile([D, NCHUNK], F32, tag="s",
                                 name="bc_ps")
                nc.tensor.matmul(bc_t[:], ones_r[:], rec_r[:],
                                 start=True, stop=True)
                nc.vector.tensor_copy(bc_s[:], bc_t[:])
            else:
                rec_d = rdram.tile([1, NCHUNK], F32, tag="rec_d")
                nc.sync.dma_start(rec_d[:], rec_f[:])
                nc.sync.dma_start(bc_s[:],
                                  rec_d[:].partition_broadcast(D))
            o_t = opool.tile([D, NCHUNK], F32, tag="o")
            nc.vector.tensor_mul(o_t[:], ot_t[0:D, :], bc_s[:])
            nc.sync.dma_start(ot_d[h][:, bass.ts(nch, NCHUNK)], o_t[:])

        def emit_s_exp(h, nch, pair):
            qt_c = qt_all[h][nch // 2][:, nch % 2, :]
            kp = kt_all[h][pair // PPIECE][:, pair % PPIECE, :]
            s_t = s_ps.tile([128, 2, NCHUNK], F32, tag="s")
            # concurrent K=64 row tiles: even m-block from
            # partitions 0-63, odd from 64-127
            nc.tensor.matmul(
                s_t[:, 0, :], kp[0:64, :], qt_c[0:64, :],
                start=True, stop=True, tile_position=(0, 0),
            )
            nc.tensor.matmul(
                s_t[:, 1, :], kp[64:128, :], qt_c[64:128, :],
                start=True, stop=True, tile_position=(64, 0),
            )
            if pair in DVE_PAIRS:
                p_i = pexp.tile([128, 2, NCHUNK], I16, tag="pi")
                nc.vector.tensor_scalar(
                    p_i[:], s_t[:], SCHRAU_A, SCHRAU_B,
                    mybir.AluOpType.mult, mybir.AluOpType.add,
                )
                return p_i[:].bitcast(BF16)
            p_t = pexp.tile([128, 2, NCHUNK], BF16, tag="pf")
            nc.scalar.activation(
                p_t[:], s_t[:],
                mybir.ActivationFunctionType.Exp,
                bias=bias_t[:], scale=1.0,
            )
            return p_t[:]

        slots = [(c, p) for c in range(NCH) for p in range(NPAIR)]
        order = []
        i1 = -(NPAIR // 2)
        for i0 in range(len(slots)):
            order.append((0,) + slots[i0])
            if 0 <= i1:
                order.append((1,) + slots[i1])
            i1 += 1
        while i1 < len(slots):
            order.append((1,) + slots[i1])
            i1 += 1

        pend = []
        for h, nch, pair in order:
            p_mm = emit_s_exp(h, nch, pair)
            pend.append((h, nch, pair, p_mm))
            if len(pend) > LAG:
                emit_o(pend.pop(0))
        while pend:
            emit_o(pend.pop(0))

    nc.compile()
    return nc


def _get_nc():
    if "nc" not in _CACHE:
        _CACHE["nc"] = _build()
    return _CACHE["nc"]


def _bf16_bits(x32):
    # round-to-nearest-even fp32 -> bf16, returned as uint16 bit payload
    u = x32.astype(np.float32).view(np.uint32)
    rounded = (u + 0x7FFF + ((u >> 16) & 1)) >> 16
    return rounded.astype(np.uint16)


def _make_in_maps(QKV):
    QKV = np.asarray(QKV, dtype=np.float32)
    q = QKV[0].reshape(HEADS, N, D)
    k = QKV[1].reshape(HEADS, N, D)
    v = QKV[2].reshape(HEADS, N, D)
    qt = q.transpose(0, 2, 1)                       # [16, 64, 4096]
    # duplicate Q^T into both 64-partition halves for the row tiles
    qt_dup = np.concatenate([qt, qt], axis=1).astype(np.float16)
    kt = k.transpose(0, 2, 1).reshape(HEADS, D, NPAIR, 2, 128)
    # even m-blocks -> partitions 0-63, odd -> 64-127, pair-major cols
    kt_packed = np.concatenate([kt[:, :, :, 0], kt[:, :, :, 1]],
                               axis=1).reshape(HEADS, 128, NPAIR * 128)
    kt_packed = kt_packed.astype(np.float16)
    v_bf = _bf16_bits(v)
    in_maps = []
    for c in range(N_CORES):
        sl = slice(c * HPC, (c + 1) * HPC)
        in_maps.append({
            "qt": np.ascontiguousarray(qt_dup[sl]),
            "kt": np.ascontiguousarray(kt_packed[sl]),
            "v": np.ascontiguousarray(v_bf[sl]),
        })
    return in_maps


def _assemble(results):
    ot = np.stack([r["ot"] for r in results])            # [8, 2, 64, 4096]
    out = ot.reshape(HEADS, D, N).transpose(0, 2, 1)     # [16, 4096, 64]
    return np.ascontiguousarray(out).reshape(B, H, N, D).astype(np.float32)


def kernel(QKV):
    nc = _get_nc()
    res = run_bass_kernel_spmd(nc, _make_in_maps(QKV), list(range(N_CORES)))
    return _assemble(res.results)


# revision 27
# speedup vs baseline: 1.0311x; 1.0311x over previous
"""Trainium2 Bass kernel for nn_BaselineAttention_25984552141259.

Problem: QKV [3, B=2, H=8, N=4096, d=64] fp32 ->
         out[b,h,n,:] = softmax(Q[b,h] @ K[b,h].T) @ V[b,h]

Sharding: B*H = 16 heads -> 2 heads per core on 8 NeuronCores.

v3 design. v1 was ACT-bound (~285us: one 128-lane 1.2 GHz activation
engine exp-ing N^2 scores) with the PE at 89%; v3 attacks both:

  1. Row-tiled S^T matmuls in fp16. d=64 contraction only fills half
     the PE array, so two m-blocks run CONCURRENTLY as K=64 tiles in
     row groups (0,0) and (64,0): K^T host-packed with even m-blocks on
     partitions 0-63 / odd on 64-127, Q^T duplicated into both halves.
     fp16 (not fp32r) because fp32r matmuls fuse their weight load and
     serialize it with the stream (measured 427ns/pair); fp16 gets
     standalone LDWEIGHTS + fast-weight-load, so pairs run
     stream-bound at ~213ns. fp16's 10 mantissa bits keep the score
     error ~4x below bf16's.
  2. exp split across engines, output in bf16. Per n-chunk the 16
     m-block pairs are exp'd 9 on ACT (table exp -> bf16) and 7 on DVE
     via Schraudolph's bit trick in bf16-bit space:
     i16 = s*(2^7/ln2) + (127*2^7 - C - 25*2^7/ln2), the int16
     reinterpreted as bf16 ~= exp(s-25) with +-3.3% sawtooth error.
     One fused tensor_scalar (mult, add, int16 convert-on-write) per
     tile, free bitcast view for the PE. Softmax renormalizes by the
     matmul'd row sums, so the approximation only redistributes
     weight; simulated end-to-end rel err 1.5e-2 vs the 2e-2 gate.
  3. O^T[d', n] = sum_m V'[m, d'] * P^T[m, n] accumulated over all 32
     m-blocks per chunk, V'/P in bf16 (V' = [V | ones] so row 64 is
     the softmax denominator), fp32 PSUM accumulation.
  4. Normalization: single-DVE-op approx reciprocal (~18 bits) on the
     denominator row, hopped through SBUF first (custom-DVE ops read
     garbage from PSUM, measured). Broadcast of 1/denom across
     partitions via DRAM-bounce stride-0 DMA so it never enters the PE
     queue.
"""
import numpy as np
from contextlib import ExitStack

import concourse.bass as bass
import concourse.tile as tile
from concourse import bacc, mybir
from concourse.bass_utils import run_bass_kernel_spmd

N_CORES = 8
B, H, N, D = 2, 8, 4096, 64
HEADS = B * H
HPC = HEADS // N_CORES          # heads per core = 2
NCHUNK = 512                    # n-tile (matmul moving free dim)
NCH = N // NCHUNK               # 8 n-chunks per head
MB = N // 128                   # 32 m-blocks of 128 keys
NPAIR = MB // 2                 # 16 row-tiled m-block pairs
PPIECE = NPAIR // 4             # pairs per K^T load piece
EXP_BIAS = -25.0

# Schraudolph exp in bf16-bit space: exp(x) ~= bitcast_bf16(
# i16(A16*x + 127*2^7 - C16)); C16 minimizes max relative error
# (+-3.3%); bias -25 folded into B16.
SCHRAU_A = 2.0 ** 7 / np.log(2.0)             # 184.6650...
SCHRAU_B = float(np.float32(127 * 2 ** 7 - 5.5 + EXP_BIAS * SCHRAU_A))
# DVE handles these m-block pairs each chunk (7/16 ~= throughput balance
# vs ACT; pattern's end-to-end rel err simulated at ~1.5e-2 max).
DVE_PAIRS = frozenset((0, 2, 4, 7, 9, 11, 14))

F32 = mybir.dt.float32
F32R = mybir.dt.float32r
F16 = mybir.dt.float16
BF16 = mybir.dt.bfloat16
I16 = mybir.dt.int16

_CACHE = {}


def _build():
    nc = bacc.Bacc("TRN2", target_bir_lowering=False, debug=False,
                   num_devices=N_CORES)
    # qt: Q^T duplicated into both 64-partition halves (row-tile moving
    # operand). kt: even m-blocks on partitions 0-63, odd on 64-127,
    # pair-major in the free dim. Both host-packed fp16.
    qt_d = nc.dram_tensor("qt", [HPC, 128, N], F16, kind="ExternalInput").ap()
    kt_d = nc.dram_tensor("kt", [HPC, 128, NPAIR * 128], F16,
                          kind="ExternalInput").ap()
    v_d = nc.dram_tensor("v", [HPC, N, D], BF16, kind="ExternalInput").ap()
    ot_d = nc.dram_tensor("ot", [HPC, D, N], F32, kind="ExternalOutput").ap()

    with tile.TileContext(nc) as tc, ExitStack() as ctx:
        const = ctx.enter_context(tc.tile_pool(name="const", bufs=1))
        qk = ctx.enter_context(tc.tile_pool(name="qk", bufs=2))
        vpool = ctx.enter_context(tc.tile_pool(name="vpool", bufs=2))
        pexp = ctx.enter_context(tc.tile_pool(name="pexp", bufs=7))
        opool = ctx.enter_context(tc.tile_pool(name="opool", bufs=3))
        rpool = ctx.enter_context(tc.tile_pool(name="rpool", bufs=2))
        s_ps = ctx.enter_context(tc.tile_pool(name="s_ps", bufs=3, space="PSUM"))
        ot_ps = ctx.enter_context(tc.tile_pool(name="ot_ps", bufs=2, space="PSUM"))
        rdram = ctx.enter_context(tc.tile_pool(name="rdram", bufs=2, space="DRAM"))

        bias_t = const.tile([128, 1], F32)
        nc.vector.memset(bias_t[:], EXP_BIAS)
        vone_f = const.tile([128, MB], BF16)
        nc.vector.memset(vone_f[:], 1.0)
        one_f = const.tile([1, 1], F32)
        nc.vector.memset(one_f[:], 1.0)
        ones_r = const.tile([1, D], F32R)
        nc.vector.tensor_copy(ones_r[:], one_f[:].to_broadcast((1, D)))

        # tiny head-start pieces: the first S-pair only needs K^T pair 0
        # (64KB) and Q^T chunk 0 (128KB) of head 0 - load those before the
        # bulk pieces so compute starts ~3us earlier.
        kt_first = qk.tile([128, 1, 128], F16, tag="ktf", name="kt_first")
        nc.sync.dma_start(
            kt_first[:],
            kt_d[0, :, 0:128].rearrange("p (t q) -> p t q", q=128))
        qt_first = qk.tile([128, 1, NCHUNK], F16, tag="qtf", name="qt_first")
        nc.sync.dma_start(
            qt_first[:],
            qt_d[0, :, 0:NCHUNK].rearrange("p (t q) -> p t q", q=NCHUNK))

        kt_all, qt_all, v_all = [], [], []
        for h in range(HPC):
            with nc.named_scope(f"load{h}"):
                # split loads so the first pairs/chunks arrive (and
                # compute starts) before the rest of the head lands
                kt_s, qt_s, v_s = [], [], []
                v_re = v_d[h].rearrange("(t p) d -> p t d", p=128)
                for i in range(4):
                    kq = qk.tile([128, PPIECE, 128], F16, tag=f"kt{i}",
                                 name=f"kt_{h}_{i}")
                    nc.sync.dma_start(
                        kq[:],
                        kt_d[h, :, bass.ts(i, PPIECE * 128)].rearrange(
                            "p (t q) -> p t q", q=128),
                    )
                    kt_s.append(kq)
                    qq = qk.tile([128, 2, NCHUNK], F16, tag=f"qt{i}",
                                 name=f"qt_{h}_{i}")
                    nc.sync.dma_start(
                        qq[:],
                        qt_d[h, :, bass.ts(i, 2 * NCHUNK)].rearrange(
                            "p (t q) -> p t q", q=NCHUNK),
                    )
                    qt_s.append(qq)
                    # V' piece [m-part, m-tile, d+1]; col 64 = 1.0 (row sums)
                    vq = vpool.tile([128, MB // 4, D + 1], BF16, tag=f"v{i}",
                                    name=f"v_{h}_{i}")
                    nc.sync.dma_start(
                        vq[:, :, 0:D],
                        v_re[:, bass.ts(i, MB // 4), :],
                    )
                    nc.vector.tensor_copy(vq[:, :, D], vone_f[:, 0:MB // 4])
                    v_s.append(vq)
                kt_all.append(kt_s)
                qt_all.append(qt_s)
                v_all.append(v_s)

        # Software-pipelined emission: the PE engine queue is strict FIFO,
        # so if O-matmuls directly follow their S-pair they stall the PE
        # for the full exp latency (~1.2us, measured as evt_wait on every
        # pair). Emitting each pair's O-matmuls LAG emissions later gives
        # the ACT/DVE exp time to land while the PE streams other pairs.
        # The two heads are interleaved as independent dependency chains
        # (fills the ~100ns/group weight-load bubbles a single chain
        # leaves), with head 1 offset by half a chunk so the chunk-end
        # normalize chains stagger and ot_ps bufs=2 never blocks.
        LAG = 10

        ot_live = {}

        def emit_o(ent):
            h, nch, pair, p_mm = ent
            if pair == 0:
                ot_live[h] = ot_ps.tile([D + 1, NCHUNK], F32, tag="ot",
                                        name=f"ot_{h}_{nch}")
            ot_t = ot_live[h]
            for j in range(2):
                m = 2 * pair + j
                nc.tensor.matmul(
                    ot_t[:],
                    v_all[h][m // (MB // 4)][:, m % (MB // 4), :],
                    p_mm[:, j, :],
                    start=(m == 0), stop=(m == MB - 1),
                )
            if pair == NPAIR - 1:
                emit_normalize(h, nch, ot_t)

        def emit_normalize(h, nch, ot_t):
            # normalize: out^T = O^T[0:64] * bcast(1 / O^T[64]).
            # single-op approx reciprocal (18 bits is plenty for a
            # softmax denominator); custom-DVE ops can't read PSUM
            # (garbage, measured) so hop the row through SBUF.
            den1 = rpool.tile([1, NCHUNK], F32, tag="den1")
            nc.scalar.copy(den1[:], ot_t[D:D + 1, :])
            rec_f = rpool.tile([1, NCHUNK], F32, tag="rec_f")
            nc.vector.reciprocal_approx_fast(rec_f[:], den1[:])
            bc_s = opool.tile([D, NCHUNK], F32, tag="bc")
            if h == HPC - 1 and nch == NCH - 1:
                # tail-only: PE K=1 broadcast matmul is ~2us faster
                # than the DRAM bounce, and at the very end the PE is
                # idle and HAM re-throttling no longer matters
                rec_r = rpool.tile([1, NCHUNK], F32R, tag="rec_r")
                nc.vector.tensor_copy(rec_r[:], rec_f[:])
                bc_t = s_ps.tile([D, NCHUNK], F32, tag="s",
                                 name="bc_ps")
                nc.tensor.matmul(bc_t[:], ones_r[:], rec_r[:],
                                 start=True, stop=True)
                nc.vector.tensor_copy(bc_s[:], bc_t[:])
            else:
                rec_d = rdram.tile([1, NCHUNK], F32, tag="rec_d")
                nc.sync.dma_start(rec_d[:], rec_f[:])
                nc.sync.dma_start(bc_s[:],
                                  rec_d[:].partition_broadcast(D))
            o_t = opool.tile([D, NCHUNK], F32, tag="o")
            nc.vector.tensor_mul(o_t[:], ot_t[0:D, :], bc_s[:])
            nc.sync.dma_start(ot_d[h][:, bass.ts(nch, NCHUNK)], o_t[:])

        def emit_s_exp(h, nch, pair):
            if h == 0 and nch == 0 and pair == 0:
                qt_c, kp = qt_first[:, 0, :], kt_first[:, 0, :]
            else:
                qt_c = qt_all[h][nch // 2][:, nch % 2, :]
                kp = kt_all[h][pair // PPIECE][:, pair % PPIECE, :]
            s_t = s_ps.tile([128, 2, NCHUNK], F32, tag="s")
            # concurrent K=64 row tiles: even m-block from
            # partitions 0-63, odd from 64-127
            nc.tensor.matmul(
                s_t[:, 0, :], kp[0:64, :], qt_c[0:64, :],
                start=True, stop=True, tile_position=(0, 0),
            )
            nc.tensor.matmul(
                s_t[:, 1, :], kp[64:128, :], qt_c[64:128, :],
                start=True, stop=True, tile_position=(64, 0),
            )
            if pair in DVE_PAIRS:
                p_i = pexp.tile([128, 2, NCHUNK], I16, tag="pi")
                nc.vector.tensor_scalar(
                    p_i[:], s_t[:], SCHRAU_A, SCHRAU_B,
                    mybir.AluOpType.mult, mybir.AluOpType.add,
                )
                return p_i[:].bitcast(BF16)
            p_t = pexp.tile([128, 2, NCHUNK], BF16, tag="pf")
            nc.scalar.activation(
                p_t[:], s_t[:],
                mybir.ActivationFunctionType.Exp,
                bias=bias_t[:], scale=1.0,
            )
            return p_t[:]

        slots = [(c, p) for c in range(NCH) for p in range(NPAIR)]
        order = []
        i1 = -(NPAIR // 2)
        for i0 in range(len(slots)):
            order.append((0,) + slots[i0])
            if 0 <= i1:
                order.append((1,) + slots[i1])
            i1 += 1
        while i1 < len(slots):
            order.append((1,) + slots[i1])
            i1 += 1

        pend = []
        for h, nch, pair in order:
            p_mm = emit_s_exp(h, nch, pair)
            pend.append((h, nch, pair, p_mm))
            if len(pend) > LAG:
                emit_o(pend.pop(0))
        while pend:
            emit_o(pend.pop(0))

    nc.compile()
    return nc


def _get_nc():
    if "nc" not in _CACHE:
        _CACHE["nc"] = _build()
    return _CACHE["nc"]


def _bf16_bits(x32):
    # round-to-nearest-even fp32 -> bf16, returned as uint16 bit payload
    u = x32.astype(np.float32).view(np.uint32)
    rounded = (u + 0x7FFF + ((u >> 16) & 1)) >> 16
    return rounded.astype(np.uint16)


def _make_in_maps(QKV):
    QKV = np.asarray(QKV, dtype=np.float32)
    q = QKV[0].reshape(HEADS, N, D)
    k = QKV[1].reshape(HEADS, N, D)
    v = QKV[2].reshape(HEADS, N, D)
    qt = q.transpose(0, 2, 1)                       # [16, 64, 4096]
    # duplicate Q^T into both 64-partition halves for the row tiles
    qt_dup = np.concatenate([qt, qt], axis=1).astype(np.float16)
    kt = k.transpose(0, 2, 1).reshape(HEADS, D, NPAIR, 2, 128)
    # even m-blocks -> partitions 0-63, odd -> 64-127, pair-major cols
    kt_packed = np.concatenate([kt[:, :, :, 0], kt[:, :, :, 1]],
                               axis=1).reshape(HEADS, 128, NPAIR * 128)
    kt_packed = kt_packed.astype(np.float16)
    v_bf = _bf16_bits(v)
    in_maps = []
    for c in range(N_CORES):
        sl = slice(c * HPC, (c + 1) * HPC)
        in_maps.append({
            "qt": np.ascontiguousarray(qt_dup[sl]),
            "kt": np.ascontiguousarray(kt_packed[sl]),
            "v": np.ascontiguousarray(v_bf[sl]),
        })
    return in_maps


def _assemble(results):
    ot = np.stack([r["ot"] for r in results])            # [8, 2, 64, 4096]
    out = ot.reshape(HEADS, D, N).transpose(0, 2, 1)     # [16, 4096, 64]
    return np.ascontiguousarray(out).reshape(B, H, N, D).astype(np.float32)


def kernel(QKV):
    nc = _get_nc()
    res = run_bass_kernel_spmd(nc, _make_in_maps(QKV), list(range(N_CORES)))
    return _assemble(res.results)


# revision 28
# speedup vs baseline: 1.2121x; 1.1756x over previous
"""Trainium2 Bass kernel for nn_BaselineAttention_25984552141259.

Problem: QKV [3, B=2, H=8, N=4096, d=64] fp32 ->
         out[b,h,n,:] = softmax(Q[b,h] @ K[b,h].T) @ V[b,h]

Sharding: B*H = 16 heads -> 2 heads per core on 8 NeuronCores.

v3 design. v1 was ACT-bound (~285us: one 128-lane 1.2 GHz activation
engine exp-ing N^2 scores) with the PE at 89%; v3 attacks both:

  1. Row-tiled S^T matmuls in fp16. d=64 contraction only fills half
     the PE array, so two m-blocks run CONCURRENTLY as K=64 tiles in
     row groups (0,0) and (64,0): K^T host-packed with even m-blocks on
     partitions 0-63 / odd on 64-127, Q^T duplicated into both halves.
     fp16 (not fp32r) because fp32r matmuls fuse their weight load and
     serialize it with the stream (measured 427ns/pair); fp16 gets
     standalone LDWEIGHTS + fast-weight-load, so pairs run
     stream-bound at ~213ns. fp16's 10 mantissa bits keep the score
     error ~4x below bf16's.
  2. exp split across engines, output in bf16. Per n-chunk the 16
     m-block pairs are exp'd 9 on ACT (table exp -> bf16) and 7 on DVE
     via Schraudolph's bit trick in bf16-bit space:
     i16 = s*(2^7/ln2) + (127*2^7 - C - 25*2^7/ln2), the int16
     reinterpreted as bf16 ~= exp(s-25) with +-3.3% sawtooth error.
     One fused tensor_scalar (mult, add, int16 convert-on-write) per
     tile, free bitcast view for the PE. Softmax renormalizes by the
     matmul'd row sums, so the approximation only redistributes
     weight; simulated end-to-end rel err 1.5e-2 vs the 2e-2 gate.
  3. O^T[d', n] = sum_m V'[m, d'] * P^T[m, n] accumulated over all 32
     m-blocks per chunk, V'/P in bf16 (V' = [V | ones] so row 64 is
     the softmax denominator), fp32 PSUM accumulation.
  4. Normalization: single-DVE-op approx reciprocal (~18 bits) on the
     denominator row, hopped through SBUF first (custom-DVE ops read
     garbage from PSUM, measured). Broadcast of 1/denom across
     partitions via DRAM-bounce stride-0 DMA so it never enters the PE
     queue.
"""
import numpy as np
from contextlib import ExitStack

import concourse.bass as bass
import concourse.tile as tile
from concourse import bacc, mybir
from concourse.bass_utils import run_bass_kernel_spmd

N_CORES = 8
B, H, N, D = 2, 8, 4096, 64
HEADS = B * H
HPC = HEADS // N_CORES          # heads per core = 2
NCHUNK = 512                    # n-tile (matmul moving free dim)
NCH = N // NCHUNK               # 8 n-chunks per head
MB = N // 128                   # 32 m-blocks of 128 keys
NPAIR = MB // 2                 # 16 row-tiled m-block pairs
PPIECE = NPAIR // 4             # pairs per K^T load piece
EXP_BIAS = -25.0

# Schraudolph exp in bf16-bit space: exp(x) ~= bitcast_bf16(
# i16(A16*x + 127*2^7 - C16)); C16 minimizes max relative error
# (+-3.3%); bias -25 folded into B16.
SCHRAU_A = 2.0 ** 7 / np.log(2.0)             # 184.6650...
SCHRAU_B = float(np.float32(127 * 2 ** 7 - 5.5 + EXP_BIAS * SCHRAU_A))
# DVE handles these m-block pairs each chunk (7/16 ~= throughput balance
# vs ACT; pattern's end-to-end rel err simulated at ~1.5e-2 max).
DVE_PAIRS = frozenset((0, 2, 4, 7, 9, 11, 14))

F32 = mybir.dt.float32
F32R = mybir.dt.float32r
F16 = mybir.dt.float16
BF16 = mybir.dt.bfloat16
I16 = mybir.dt.int16

_CACHE = {}


def _build():
    nc = bacc.Bacc("TRN2", target_bir_lowering=False, debug=False,
                   num_devices=N_CORES)
    # qt: Q^T duplicated into both 64-partition halves (row-tile moving
    # operand). kt: even m-blocks on partitions 0-63, odd on 64-127,
    # pair-major in the free dim. Both host-packed fp16.
    qt_d = nc.dram_tensor("qt", [HPC, 128, N], F16, kind="ExternalInput").ap()
    kt_d = nc.dram_tensor("kt", [HPC, 128, NPAIR * 128], F16,
                          kind="ExternalInput").ap()
    v_d = nc.dram_tensor("v", [HPC, N, D], BF16, kind="ExternalInput").ap()
    ot_d = nc.dram_tensor("ot", [HPC, D, N], F32, kind="ExternalOutput").ap()

    with tile.TileContext(nc) as tc, ExitStack() as ctx:
        const = ctx.enter_context(tc.tile_pool(name="const", bufs=1))
        qk = ctx.enter_context(tc.tile_pool(name="qk", bufs=2))
        vpool = ctx.enter_context(tc.tile_pool(name="vpool", bufs=2))
        pexp = ctx.enter_context(tc.tile_pool(name="pexp", bufs=5))
        opool = ctx.enter_context(tc.tile_pool(name="opool", bufs=3))
        rpool = ctx.enter_context(tc.tile_pool(name="rpool", bufs=2))
        s_ps = ctx.enter_context(tc.tile_pool(name="s_ps", bufs=3, space="PSUM"))
        ot_ps = ctx.enter_context(tc.tile_pool(name="ot_ps", bufs=2, space="PSUM"))
        rdram = ctx.enter_context(tc.tile_pool(name="rdram", bufs=2, space="DRAM"))

        bias_t = const.tile([128, 1], F32)
        nc.vector.memset(bias_t[:], EXP_BIAS)
        vone_f = const.tile([128, MB], BF16)
        nc.vector.memset(vone_f[:], 1.0)
        one_f = const.tile([1, 1], F32)
        nc.vector.memset(one_f[:], 1.0)
        ones_r = const.tile([1, D], F32R)
        nc.vector.tensor_copy(ones_r[:], one_f[:].to_broadcast((1, D)))

        kt_all, qt_all, v_all = [], [], []
        for h in range(HPC):
            with nc.named_scope(f"load{h}"):
                # split loads so the first pairs/chunks arrive (and
                # compute starts) before the rest of the head lands
                kt_s, qt_s, v_s = [], [], []
                v_re = v_d[h].rearrange("(t p) d -> p t d", p=128)
                for i in range(4):
                    kq = qk.tile([128, PPIECE, 128], F16, tag=f"kt{i}",
                                 name=f"kt_{h}_{i}")
                    nc.sync.dma_start(
                        kq[:],
                        kt_d[h, :, bass.ts(i, PPIECE * 128)].rearrange(
                            "p (t q) -> p t q", q=128),
                    )
                    kt_s.append(kq)
                    qq = qk.tile([128, 2, NCHUNK], F16, tag=f"qt{i}",
                                 name=f"qt_{h}_{i}")
                    nc.sync.dma_start(
                        qq[:],
                        qt_d[h, :, bass.ts(i, 2 * NCHUNK)].rearrange(
                            "p (t q) -> p t q", q=NCHUNK),
                    )
                    qt_s.append(qq)
                    # V' piece [m-part, m-tile, d+1]; col 64 = 1.0 (row sums)
                    vq = vpool.tile([128, MB // 4, D + 1], BF16, tag=f"v{i}",
                                    name=f"v_{h}_{i}")
                    nc.sync.dma_start(
                        vq[:, :, 0:D],
                        v_re[:, bass.ts(i, MB // 4), :],
                    )
                    nc.vector.tensor_copy(vq[:, :, D], vone_f[:, 0:MB // 4])
                    v_s.append(vq)
                kt_all.append(kt_s)
                qt_all.append(qt_s)
                v_all.append(v_s)

        # Software-pipelined emission: the PE engine queue is strict FIFO,
        # so if O-matmuls directly follow their S-pair they stall the PE
        # for the full exp latency (~1.2us, measured as evt_wait on every
        # pair). Emitting each pair's O-matmuls LAG emissions later gives
        # the ACT/DVE exp time to land while the PE streams other pairs.
        # The two heads are interleaved as independent dependency chains
        # (fills the ~100ns/group weight-load bubbles a single chain
        # leaves), with head 1 offset by half a chunk so the chunk-end
        # normalize chains stagger and ot_ps bufs=2 never blocks.
        LAG = 6

        ot_live = {}

        def emit_o(ent):
            h, nch, pair, p_mm = ent
            if pair == 0:
                ot_live[h] = ot_ps.tile([D + 1, NCHUNK], F32, tag="ot",
                                        name=f"ot_{h}_{nch}")
            ot_t = ot_live[h]
            for j in range(2):
                m = 2 * pair + j
                nc.tensor.matmul(
                    ot_t[:],
                    v_all[h][m // (MB // 4)][:, m % (MB // 4), :],
                    p_mm[:, j, :],
                    start=(m == 0), stop=(m == MB - 1),
                )
            if pair == NPAIR - 1:
                emit_normalize(h, nch, ot_t)

        def emit_normalize(h, nch, ot_t):
            # normalize: out^T = O^T[0:64] * bcast(1 / O^T[64]).
            # single-op approx reciprocal (18 bits is plenty for a
            # softmax denominator); custom-DVE ops can't read PSUM
            # (garbage, measured) so hop the row through SBUF.
            den1 = rpool.tile([1, NCHUNK], F32, tag="den1")
            nc.scalar.copy(den1[:], ot_t[D:D + 1, :])
            rec_f = rpool.tile([1, NCHUNK], F32, tag="rec_f")
            nc.vector.reciprocal_approx_fast(rec_f[:], den1[:])
            bc_s = opool.tile([D, NCHUNK], F32, tag="bc")
            if h == HPC - 1 and nch == NCH - 1:
                # tail-only: PE K=1 broadcast matmul is ~2us faster
                # than the DRAM bounce, and at the very end the PE is
                # idle and HAM re-throttling no longer matters
                rec_r = rpool.tile([1, NCHUNK], F32R, tag="rec_r")
                nc.vector.tensor_copy(rec_r[:], rec_f[:])
                bc_t = s_ps.tile([D, NCHUNK], F32, tag="s",
                                 name="bc_ps")
                nc.tensor.matmul(bc_t[:], ones_r[:], rec_r[:],
                                 start=True, stop=True)
                nc.vector.tensor_copy(bc_s[:], bc_t[:])
            else:
                rec_d = rdram.tile([1, NCHUNK], F32, tag="rec_d")
                nc.sync.dma_start(rec_d[:], rec_f[:])
                nc.sync.dma_start(bc_s[:],
                                  rec_d[:].partition_broadcast(D))
            o_t = opool.tile([D, NCHUNK], F32, tag="o")
            nc.vector.tensor_mul(o_t[:], ot_t[0:D, :], bc_s[:])
            nc.sync.dma_start(ot_d[h][:, bass.ts(nch, NCHUNK)], o_t[:])

        def emit_s_exp(h, nch, pair):
            qt_c = qt_all[h][nch // 2][:, nch % 2, :]
            kp = kt_all[h][pair // PPIECE][:, pair % PPIECE, :]
            s_t = s_ps.tile([128, 2, NCHUNK], F32, tag="s")
            # concurrent K=64 row tiles: even m-block from
            # partitions 0-63, odd from 64-127
            nc.tensor.matmul(
                s_t[:, 0, :], kp[0:64, :], qt_c[0:64, :],
                start=True, stop=True, tile_position=(0, 0),
            )
            nc.tensor.matmul(
                s_t[:, 1, :], kp[64:128, :], qt_c[64:128, :],
                start=True, stop=True, tile_position=(64, 0),
            )
            if pair in DVE_PAIRS:
                p_i = pexp.tile([128, 2, NCHUNK], I16, tag="pi")
                nc.vector.tensor_scalar(
                    p_i[:], s_t[:], SCHRAU_A, SCHRAU_B,
                    mybir.AluOpType.mult, mybir.AluOpType.add,
                )
                return p_i[:].bitcast(BF16)
            p_t = pexp.tile([128, 2, NCHUNK], BF16, tag="pf")
            nc.scalar.activation(
                p_t[:], s_t[:],
                mybir.ActivationFunctionType.Exp,
                bias=bias_t[:], scale=1.0,
            )
            return p_t[:]

        slots = [(c, p) for c in range(NCH) for p in range(NPAIR)]
        order = []
        i1 = -(NPAIR // 2)
        for i0 in range(len(slots)):
            order.append((0,) + slots[i0])
            if 0 <= i1:
                order.append((1,) + slots[i1])
            i1 += 1
        while i1 < len(slots):
            order.append((1,) + slots[i1])
            i1 += 1

        pend = []
        for h, nch, pair in order:
            p_mm = emit_s_exp(h, nch, pair)
            pend.append((h, nch, pair, p_mm))
            if len(pend) > LAG:
                emit_o(pend.pop(0))
        while pend:
            emit_o(pend.pop(0))

    nc.compile()
    return nc


def _get_nc():
    if "nc" not in _CACHE:
        _CACHE["nc"] = _build()
    return _CACHE["nc"]


def _bf16_bits(x32):
    # round-to-nearest-even fp32 -> bf16, returned as uint16 bit payload
    u = x32.astype(np.float32).view(np.uint32)
    rounded = (u + 0x7FFF + ((u >> 16) & 1)) >> 16
    return rounded.astype(np.uint16)


def _make_in_maps(QKV):
    QKV = np.asarray(QKV, dtype=np.float32)
    q = QKV[0].reshape(HEADS, N, D)
    k = QKV[1].reshape(HEADS, N, D)
    v = QKV[2].reshape(HEADS, N, D)
    qt = q.transpose(0, 2, 1)                       # [16, 64, 4096]
    # duplicate Q^T into both 64-partition halves for the row tiles
    qt_dup = np.concatenate([qt, qt], axis=1).astype(np.float16)
    kt = k.transpose(0, 2, 1).reshape(HEADS, D, NPAIR, 2, 128)
    # even m-blocks -> partitions 0-63, odd -> 64-127, pair-major cols
    kt_packed = np.concatenate([kt[:, :, :, 0], kt[:, :, :, 1]],
                               axis=1).reshape(HEADS, 128, NPAIR * 128)
    kt_packed = kt_packed.astype(np.float16)
    v_bf = _bf16_bits(v)
    in_maps = []
    for c in range(N_CORES):
        sl = slice(c * HPC, (c + 1) * HPC)
        in_maps.append({
            "qt": np.ascontiguousarray(qt_dup[sl]),
            "kt": np.ascontiguousarray(kt_packed[sl]),
            "v": np.ascontiguousarray(v_bf[sl]),
        })
    return in_maps


def _assemble(results):
    ot = np.stack([r["ot"] for r in results])            # [8, 2, 64, 4096]
    out = ot.reshape(HEADS, D, N).transpose(0, 2, 1)     # [16, 4096, 64]
    return np.ascontiguousarray(out).reshape(B, H, N, D).astype(np.float32)


def kernel(QKV):
    nc = _get_nc()
    res = run_bass_kernel_spmd(nc, _make_in_maps(QKV), list(range(N_CORES)))
    return _assemble(res.results)


# revision 29
# speedup vs baseline: 1.2211x; 1.0074x over previous
"""Trainium2 Bass kernel for nn_BaselineAttention_25984552141259.

Problem: QKV [3, B=2, H=8, N=4096, d=64] fp32 ->
         out[b,h,n,:] = softmax(Q[b,h] @ K[b,h].T) @ V[b,h]

Sharding: B*H = 16 heads -> 2 heads per core on 8 NeuronCores.

v3 design. v1 was ACT-bound (~285us: one 128-lane 1.2 GHz activation
engine exp-ing N^2 scores) with the PE at 89%; v3 attacks both:

  1. Row-tiled S^T matmuls in fp16. d=64 contraction only fills half
     the PE array, so two m-blocks run CONCURRENTLY as K=64 tiles in
     row groups (0,0) and (64,0): K^T host-packed with even m-blocks on
     partitions 0-63 / odd on 64-127, Q^T duplicated into both halves.
     fp16 (not fp32r) because fp32r matmuls fuse their weight load and
     serialize it with the stream (measured 427ns/pair); fp16 gets
     standalone LDWEIGHTS + fast-weight-load, so pairs run
     stream-bound at ~213ns. fp16's 10 mantissa bits keep the score
     error ~4x below bf16's.
  2. exp split across engines, output in bf16. Per n-chunk the 16
     m-block pairs are exp'd 9 on ACT (table exp -> bf16) and 7 on DVE
     via Schraudolph's bit trick in bf16-bit space:
     i16 = s*(2^7/ln2) + (127*2^7 - C - 25*2^7/ln2), the int16
     reinterpreted as bf16 ~= exp(s-25) with +-3.3% sawtooth error.
     One fused tensor_scalar (mult, add, int16 convert-on-write) per
     tile, free bitcast view for the PE. Softmax renormalizes by the
     matmul'd row sums, so the approximation only redistributes
     weight; simulated end-to-end rel err 1.5e-2 vs the 2e-2 gate.
  3. O^T[d', n] = sum_m V'[m, d'] * P^T[m, n] accumulated over all 32
     m-blocks per chunk, V'/P in bf16 (V' = [V | ones] so row 64 is
     the softmax denominator), fp32 PSUM accumulation.
  4. Normalization: single-DVE-op approx reciprocal (~18 bits) on the
     denominator row, hopped through SBUF first (custom-DVE ops read
     garbage from PSUM, measured). Broadcast of 1/denom across
     partitions via DRAM-bounce stride-0 DMA so it never enters the PE
     queue.
"""
import numpy as np
from contextlib import ExitStack

import concourse.bass as bass
import concourse.tile as tile
from concourse import bacc, mybir
from concourse.bass_utils import run_bass_kernel_spmd

N_CORES = 8
B, H, N, D = 2, 8, 4096, 64
HEADS = B * H
HPC = HEADS // N_CORES          # heads per core = 2
NCHUNK = 512                    # n-tile (matmul moving free dim)
NCH = N // NCHUNK               # 8 n-chunks per head
MB = N // 128                   # 32 m-blocks of 128 keys
NPAIR = MB // 2                 # 16 row-tiled m-block pairs
PPIECE = NPAIR // 4             # pairs per K^T load piece
EXP_BIAS = -25.0

# Schraudolph exp in bf16-bit space: exp(x) ~= bitcast_bf16(
# i16(A16*x + 127*2^7 - C16)); C16 minimizes max relative error
# (+-3.3%); bias -25 folded into B16.
SCHRAU_A = 2.0 ** 7 / np.log(2.0)             # 184.6650...
SCHRAU_B = float(np.float32(127 * 2 ** 7 - 5.5 + EXP_BIAS * SCHRAU_A))
# DVE handles these m-block pairs each chunk (7/16 ~= throughput balance
# vs ACT; pattern's end-to-end rel err simulated at ~1.5e-2 max).
DVE_PAIRS = frozenset((0, 2, 4, 7, 9, 11, 14))

F32 = mybir.dt.float32
F32R = mybir.dt.float32r
F16 = mybir.dt.float16
BF16 = mybir.dt.bfloat16
I16 = mybir.dt.int16

_CACHE = {}


def _build():
    nc = bacc.Bacc("TRN2", target_bir_lowering=False, debug=False,
                   num_devices=N_CORES)
    # qt: Q^T duplicated into both 64-partition halves (row-tile moving
    # operand). kt: even m-blocks on partitions 0-63, odd on 64-127,
    # pair-major in the free dim. Both host-packed fp16.
    qt_d = nc.dram_tensor("qt", [HPC, 128, N], F16, kind="ExternalInput").ap()
    kt_d = nc.dram_tensor("kt", [HPC, 128, NPAIR * 128], F16,
                          kind="ExternalInput").ap()
    v_d = nc.dram_tensor("v", [HPC, N, D], BF16, kind="ExternalInput").ap()
    ot_d = nc.dram_tensor("ot", [HPC, D, N], F32, kind="ExternalOutput").ap()

    with tile.TileContext(nc) as tc, ExitStack() as ctx:
        const = ctx.enter_context(tc.tile_pool(name="const", bufs=1))
        qk = ctx.enter_context(tc.tile_pool(name="qk", bufs=2))
        vpool = ctx.enter_context(tc.tile_pool(name="vpool", bufs=2))
        pexp = ctx.enter_context(tc.tile_pool(name="pexp", bufs=5))
        opool = ctx.enter_context(tc.tile_pool(name="opool", bufs=3))
        rpool = ctx.enter_context(tc.tile_pool(name="rpool", bufs=2))
        s_ps = ctx.enter_context(tc.tile_pool(name="s_ps", bufs=3, space="PSUM"))
        ot_ps = ctx.enter_context(tc.tile_pool(name="ot_ps", bufs=2, space="PSUM"))
        rdram = ctx.enter_context(tc.tile_pool(name="rdram", bufs=2, space="DRAM"))

        bias_t = const.tile([128, 1], F32)
        nc.vector.memset(bias_t[:], EXP_BIAS)
        vone_f = const.tile([128, MB], BF16)
        nc.vector.memset(vone_f[:], 1.0)
        one_f = const.tile([1, 1], F32)
        nc.vector.memset(one_f[:], 1.0)
        ones_r = const.tile([1, D], F32R)
        nc.vector.tensor_copy(ones_r[:], one_f[:].to_broadcast((1, D)))

        kt_all, qt_all, v_all = [], [], []
        for h in range(HPC):
            with nc.named_scope(f"load{h}"):
                # split loads so the first pairs/chunks arrive (and
                # compute starts) before the rest of the head lands
                kt_s, qt_s, v_s = [], [], []
                v_re = v_d[h].rearrange("(t p) d -> p t d", p=128)
                for i in range(4):
                    kq = qk.tile([128, PPIECE, 128], F16, tag=f"kt{i}",
                                 name=f"kt_{h}_{i}")
                    nc.sync.dma_start(
                        kq[:],
                        kt_d[h, :, bass.ts(i, PPIECE * 128)].rearrange(
                            "p (t q) -> p t q", q=128),
                    )
                    kt_s.append(kq)
                    qq = qk.tile([128, 2, NCHUNK], F16, tag=f"qt{i}",
                                 name=f"qt_{h}_{i}")
                    nc.sync.dma_start(
                        qq[:],
                        qt_d[h, :, bass.ts(i, 2 * NCHUNK)].rearrange(
                            "p (t q) -> p t q", q=NCHUNK),
                    )
                    qt_s.append(qq)
                    # V' piece [m-part, m-tile, d+1]; col 64 = 1.0 (row sums)
                    vq = vpool.tile([128, MB // 4, D + 1], BF16, tag=f"v{i}",
                                    name=f"v_{h}_{i}")
                    nc.sync.dma_start(
                        vq[:, :, 0:D],
                        v_re[:, bass.ts(i, MB // 4), :],
                    )
                    nc.vector.tensor_copy(vq[:, :, D], vone_f[:, 0:MB // 4])
                    v_s.append(vq)
                kt_all.append(kt_s)
                qt_all.append(qt_s)
                v_all.append(v_s)

        # Software-pipelined emission: the PE engine queue is strict FIFO,
        # so if O-matmuls directly follow their S-pair they stall the PE
        # for the full exp latency (~1.2us, measured as evt_wait on every
        # pair). Emitting each pair's O-matmuls LAG emissions later gives
        # the ACT/DVE exp time to land while the PE streams other pairs.
        # The two heads are interleaved as independent dependency chains
        # (fills the ~100ns/group weight-load bubbles a single chain
        # leaves), with head 1 offset by half a chunk so the chunk-end
        # normalize chains stagger and ot_ps bufs=2 never blocks.
        LAG = 6

        ot_live = {}

        def emit_o(ent):
            h, nch, pair, p_mm = ent
            if pair == 0:
                ot_live[h] = ot_ps.tile([D + 1, NCHUNK], F32, tag="ot",
                                        name=f"ot_{h}_{nch}")
            ot_t = ot_live[h]
            for j in range(2):
                m = 2 * pair + j
                nc.tensor.matmul(
                    ot_t[:],
                    v_all[h][m // (MB // 4)][:, m % (MB // 4), :],
                    p_mm[:, j, :],
                    start=(m == 0), stop=(m == MB - 1),
                )
            if pair == NPAIR - 1:
                emit_normalize(h, nch, ot_t)

        def emit_normalize(h, nch, ot_t):
            # normalize: out^T = O^T[0:64] * bcast(1 / O^T[64]).
            # single-op approx reciprocal (18 bits is plenty for a
            # softmax denominator); custom-DVE ops can't read PSUM
            # (garbage, measured) so hop the row through SBUF.
            den1 = rpool.tile([1, NCHUNK], F32, tag="den1")
            nc.scalar.copy(den1[:], ot_t[D:D + 1, :])
            rec_f = rpool.tile([1, NCHUNK], F32, tag="rec_f")
            nc.vector.reciprocal_approx_fast(rec_f[:], den1[:])
            bc_s = opool.tile([D, NCHUNK], F32, tag="bc")
            if h == HPC - 1 and nch == NCH - 1:
                # tail-only: PE K=1 broadcast matmul is ~2us faster
                # than the DRAM bounce, and at the very end the PE is
                # idle and HAM re-throttling no longer matters
                rec_r = rpool.tile([1, NCHUNK], F32R, tag="rec_r")
                nc.vector.tensor_copy(rec_r[:], rec_f[:])
                bc_t = s_ps.tile([D, NCHUNK], F32, tag="s",
                                 name="bc_ps")
                nc.tensor.matmul(bc_t[:], ones_r[:], rec_r[:],
                                 start=True, stop=True)
                nc.vector.tensor_copy(bc_s[:], bc_t[:])
            else:
                rec_d = rdram.tile([1, NCHUNK], F32, tag="rec_d")
                nc.sync.dma_start(rec_d[:], rec_f[:])
                nc.sync.dma_start(bc_s[:],
                                  rec_d[:].partition_broadcast(D))
            o_t = opool.tile([D, NCHUNK], F32, tag="o")
            nc.vector.tensor_mul(o_t[:], ot_t[0:D, :], bc_s[:])
            nc.sync.dma_start(ot_d[h][:, bass.ts(nch, NCHUNK)], o_t[:])

        def emit_s_exp(h, nch, pair):
            qt_c = qt_all[h][nch // 2][:, nch % 2, :]
            kp = kt_all[h][pair // PPIECE][:, pair % PPIECE, :]
            s_t = s_ps.tile([128, 2, NCHUNK], F32, tag="s")
            # concurrent K=64 row tiles: even m-block from
            # partitions 0-63, odd from 64-127
            nc.tensor.matmul(
                s_t[:, 0, :], kp[0:64, :], qt_c[0:64, :],
                start=True, stop=True, tile_position=(0, 0),
            )
            nc.tensor.matmul(
                s_t[:, 1, :], kp[64:128, :], qt_c[64:128, :],
                start=True, stop=True, tile_position=(64, 0),
            )
            if pair in DVE_PAIRS:
                p_i = pexp.tile([128, 2, NCHUNK], I16, tag="pi")
                nc.vector.tensor_scalar(
                    p_i[:], s_t[:], SCHRAU_A, SCHRAU_B,
                    mybir.AluOpType.mult, mybir.AluOpType.add,
                )
                return p_i[:].bitcast(BF16)
            p_t = pexp.tile([128, 2, NCHUNK], BF16, tag="pf")
            nc.scalar.activation(
                p_t[:], s_t[:],
                mybir.ActivationFunctionType.Exp,
                bias=bias_t[:], scale=1.0,
            )
            return p_t[:]

        # Emission in blocks of 2 same-head pairs: the PE pays ~100ns per
        # tile-geometry switch between (64,128) S-tiles and (128,128)
        # O-tiles (measured: the second V' load of a group hides in the
        # background weight buffer, the cross-geometry ones don't).
        # Batching 2 S-pairs then 4 O-matmuls halves the switch count.
        slots = [(c, p) for c in range(NCH) for p in range(NPAIR)]
        h_blocks = [slots[i:i + 2] for i in range(0, len(slots), 2)]
        order_blocks = []
        j = -(NPAIR // 4)
        for i in range(len(h_blocks)):
            order_blocks.append((0, h_blocks[i]))
            if j >= 0:
                order_blocks.append((1, h_blocks[j]))
            j += 1
        while j < len(h_blocks):
            if j >= 0:
                order_blocks.append((1, h_blocks[j]))
            j += 1

        pend = []
        for h, blk in order_blocks:
            for c, p in blk:
                p_mm = emit_s_exp(h, c, p)
                pend.append((h, c, p, p_mm))
            while len(pend) > LAG:
                emit_o(pend.pop(0))
        while pend:
            emit_o(pend.pop(0))

    nc.compile()
    return nc


def _get_nc():
    if "nc" not in _CACHE:
        _CACHE["nc"] = _build()
    return _CACHE["nc"]


def _bf16_bits(x32):
    # round-to-nearest-even fp32 -> bf16, returned as uint16 bit payload
    u = x32.astype(np.float32).view(np.uint32)
    rounded = (u + 0x7FFF + ((u >> 16) & 1)) >> 16
    return rounded.astype(np.uint16)


def _make_in_maps(QKV):
    QKV = np.asarray(QKV, dtype=np.float32)
    q = QKV[0].reshape(HEADS, N, D)
    k = QKV[1].reshape(HEADS, N, D)
    v = QKV[2].reshape(HEADS, N, D)
    qt = q.transpose(0, 2, 1)                       # [16, 64, 4096]
    # duplicate Q^T into both 64-partition halves for the row tiles
    qt_dup = np.concatenate([qt, qt], axis=1).astype(np.float16)
    kt = k.transpose(0, 2, 1).reshape(HEADS, D, NPAIR, 2, 128)
    # even m-blocks -> partitions 0-63, odd -> 64-127, pair-major cols
    kt_packed = np.concatenate([kt[:, :, :, 0], kt[:, :, :, 1]],
                               axis=1).reshape(HEADS, 128, NPAIR * 128)
    kt_packed = kt_packed.astype(np.float16)
    v_bf = _bf16_bits(v)
    in_maps = []
    for c in range(N_CORES):
        sl = slice(c * HPC, (c + 1) * HPC)
        in_maps.append({
            "qt": np.ascontiguousarray(qt_dup[sl]),
            "kt": np.ascontiguousarray(kt_packed[sl]),
            "v": np.ascontiguousarray(v_bf[sl]),
        })
    return in_maps


def _assemble(results):
    ot = np.stack([r["ot"] for r in results])            # [8, 2, 64, 4096]
    out = ot.reshape(HEADS, D, N).transpose(0, 2, 1)     # [16, 4096, 64]
    return np.ascontiguousarray(out).reshape(B, H, N, D).astype(np.float32)


def kernel(QKV):
    nc = _get_nc()
    res = run_bass_kernel_spmd(nc, _make_in_maps(QKV), list(range(N_CORES)))
    return _assemble(res.results)


# revision 30
# speedup vs baseline: 1.2346x; 1.0111x over previous
"""Trainium2 Bass kernel for nn_BaselineAttention_25984552141259.

Problem: QKV [3, B=2, H=8, N=4096, d=64] fp32 ->
         out[b,h,n,:] = softmax(Q[b,h] @ K[b,h].T) @ V[b,h]

Sharding: B*H = 16 heads -> 2 heads per core on 8 NeuronCores.

v3 design. v1 was ACT-bound (~285us: one 128-lane 1.2 GHz activation
engine exp-ing N^2 scores) with the PE at 89%; v3 attacks both:

  1. Row-tiled S^T matmuls in fp16. d=64 contraction only fills half
     the PE array, so two m-blocks run CONCURRENTLY as K=64 tiles in
     row groups (0,0) and (64,0): K^T host-packed with even m-blocks on
     partitions 0-63 / odd on 64-127, Q^T duplicated into both halves.
     fp16 (not fp32r) because fp32r matmuls fuse their weight load and
     serialize it with the stream (measured 427ns/pair); fp16 gets
     standalone LDWEIGHTS + fast-weight-load, so pairs run
     stream-bound at ~213ns. fp16's 10 mantissa bits keep the score
     error ~4x below bf16's.
  2. exp split across engines, output in bf16. Per n-chunk the 16
     m-block pairs are exp'd 9 on ACT (table exp -> bf16) and 7 on DVE
     via Schraudolph's bit trick in bf16-bit space:
     i16 = s*(2^7/ln2) + (127*2^7 - C - 25*2^7/ln2), the int16
     reinterpreted as bf16 ~= exp(s-25) with +-3.3% sawtooth error.
     One fused tensor_scalar (mult, add, int16 convert-on-write) per
     tile, free bitcast view for the PE. Softmax renormalizes by the
     matmul'd row sums, so the approximation only redistributes
     weight; simulated end-to-end rel err 1.5e-2 vs the 2e-2 gate.
  3. O^T[d', n] = sum_m V'[m, d'] * P^T[m, n] accumulated over all 32
     m-blocks per chunk, V'/P in bf16 (V' = [V | ones] so row 64 is
     the softmax denominator), fp32 PSUM accumulation.
  4. Normalization: single-DVE-op approx reciprocal (~18 bits) on the
     denominator row, hopped through SBUF first (custom-DVE ops read
     garbage from PSUM, measured). Broadcast of 1/denom across
     partitions via DRAM-bounce stride-0 DMA so it never enters the PE
     queue.
"""
import numpy as np
from contextlib import ExitStack

import concourse.bass as bass
import concourse.tile as tile
from concourse import bacc, mybir
from concourse.bass_utils import run_bass_kernel_spmd

N_CORES = 8
B, H, N, D = 2, 8, 4096, 64
HEADS = B * H
HPC = HEADS // N_CORES          # heads per core = 2
NCHUNK = 512                    # n-tile (matmul moving free dim)
NCH = N // NCHUNK               # 8 n-chunks per head
MB = N // 128                   # 32 m-blocks of 128 keys
NPAIR = MB // 2                 # 16 row-tiled m-block pairs
PPIECE = NPAIR // 4             # pairs per K^T load piece
EXP_BIAS = -25.0

# Schraudolph exp in bf16-bit space: exp(x) ~= bitcast_bf16(
# i16(A16*x + 127*2^7 - C16)); C16 minimizes max relative error
# (+-3.3%); bias -25 folded into B16.
SCHRAU_A = 2.0 ** 7 / np.log(2.0)             # 184.6650...
SCHRAU_B = float(np.float32(127 * 2 ** 7 - 5.5 + EXP_BIAS * SCHRAU_A))
# DVE handles these m-block pairs each chunk (7/16 ~= throughput balance
# vs ACT; pattern's end-to-end rel err simulated at ~1.5e-2 max).
DVE_PAIRS = frozenset((0, 2, 4, 7, 9, 11, 14))

F32 = mybir.dt.float32
F32R = mybir.dt.float32r
F16 = mybir.dt.float16
BF16 = mybir.dt.bfloat16
I16 = mybir.dt.int16

_CACHE = {}


def _build():
    nc = bacc.Bacc("TRN2", target_bir_lowering=False, debug=False,
                   num_devices=N_CORES)
    # qt: Q^T duplicated into both 64-partition halves (row-tile moving
    # operand). kt: even m-blocks on partitions 0-63, odd on 64-127,
    # pair-major in the free dim. Both host-packed fp16.
    qt_d = nc.dram_tensor("qt", [HPC, 128, N], F16, kind="ExternalInput").ap()
    kt_d = nc.dram_tensor("kt", [HPC, 128, NPAIR * 128], F16,
                          kind="ExternalInput").ap()
    v_d = nc.dram_tensor("v", [HPC, N, D], BF16, kind="ExternalInput").ap()
    ot_d = nc.dram_tensor("ot", [HPC, D, N], F32, kind="ExternalOutput").ap()

    with tile.TileContext(nc) as tc, ExitStack() as ctx:
        const = ctx.enter_context(tc.tile_pool(name="const", bufs=1))
        qk = ctx.enter_context(tc.tile_pool(name="qk", bufs=2))
        vpool = ctx.enter_context(tc.tile_pool(name="vpool", bufs=2))
        pexp = ctx.enter_context(tc.tile_pool(name="pexp", bufs=6))
        opool = ctx.enter_context(tc.tile_pool(name="opool", bufs=3))
        rpool = ctx.enter_context(tc.tile_pool(name="rpool", bufs=2))
        s_ps = ctx.enter_context(tc.tile_pool(name="s_ps", bufs=3, space="PSUM"))
        ot_ps = ctx.enter_context(tc.tile_pool(name="ot_ps", bufs=2, space="PSUM"))
        rdram = ctx.enter_context(tc.tile_pool(name="rdram", bufs=2, space="DRAM"))

        bias_t = const.tile([128, 1], F32)
        nc.vector.memset(bias_t[:], EXP_BIAS)
        vone_f = const.tile([128, MB], BF16)
        nc.vector.memset(vone_f[:], 1.0)
        one_f = const.tile([1, 1], F32)
        nc.vector.memset(one_f[:], 1.0)
        ones_r = const.tile([1, D], F32R)
        nc.vector.tensor_copy(ones_r[:], one_f[:].to_broadcast((1, D)))

        kt_all, qt_all, v_all = [], [], []
        for h in range(HPC):
            with nc.named_scope(f"load{h}"):
                # split loads so the first pairs/chunks arrive (and
                # compute starts) before the rest of the head lands
                kt_s, qt_s, v_s = [], [], []
                v_re = v_d[h].rearrange("(t p) d -> p t d", p=128)
                for i in range(4):
                    kq = qk.tile([128, PPIECE, 128], F16, tag=f"kt{i}",
                                 name=f"kt_{h}_{i}")
                    nc.sync.dma_start(
                        kq[:],
                        kt_d[h, :, bass.ts(i, PPIECE * 128)].rearrange(
                            "p (t q) -> p t q", q=128),
                    )
                    kt_s.append(kq)
                    qq = qk.tile([128, 2, NCHUNK], F16, tag=f"qt{i}",
                                 name=f"qt_{h}_{i}")
                    nc.sync.dma_start(
                        qq[:],
                        qt_d[h, :, bass.ts(i, 2 * NCHUNK)].rearrange(
                            "p (t q) -> p t q", q=NCHUNK),
                    )
                    qt_s.append(qq)
                    # V' piece [m-part, m-tile, d+1]; col 64 = 1.0 (row sums)
                    vq = vpool.tile([128, MB // 4, D + 1], BF16, tag=f"v{i}",
                                    name=f"v_{h}_{i}")
                    nc.sync.dma_start(
                        vq[:, :, 0:D],
                        v_re[:, bass.ts(i, MB // 4), :],
                    )
                    nc.vector.tensor_copy(vq[:, :, D], vone_f[:, 0:MB // 4])
                    v_s.append(vq)
                kt_all.append(kt_s)
                qt_all.append(qt_s)
                v_all.append(v_s)

        # Software-pipelined emission: the PE engine queue is strict FIFO,
        # so if O-matmuls directly follow their S-pair they stall the PE
        # for the full exp latency (~1.2us, measured as evt_wait on every
        # pair). Emitting each pair's O-matmuls LAG emissions later gives
        # the ACT/DVE exp time to land while the PE streams other pairs.
        # The two heads are interleaved as independent dependency chains
        # (fills the ~100ns/group weight-load bubbles a single chain
        # leaves), with head 1 offset by half a chunk so the chunk-end
        # normalize chains stagger and ot_ps bufs=2 never blocks.
        LAG = 8

        ot_live = {}

        def emit_o(ent):
            h, nch, pair, p_mm = ent
            if pair == 0:
                ot_live[h] = ot_ps.tile([D + 1, NCHUNK], F32, tag="ot",
                                        name=f"ot_{h}_{nch}")
            ot_t = ot_live[h]
            for j in range(2):
                m = 2 * pair + j
                nc.tensor.matmul(
                    ot_t[:],
                    v_all[h][m // (MB // 4)][:, m % (MB // 4), :],
                    p_mm[:, j, :],
                    start=(m == 0), stop=(m == MB - 1),
                )
            if pair == NPAIR - 1:
                emit_normalize(h, nch, ot_t)

        def emit_normalize(h, nch, ot_t):
            # normalize: out^T = O^T[0:64] * bcast(1 / O^T[64]).
            # single-op approx reciprocal (18 bits is plenty for a
            # softmax denominator); custom-DVE ops can't read PSUM
            # (garbage, measured) so hop the row through SBUF.
            den1 = rpool.tile([1, NCHUNK], F32, tag="den1")
            nc.scalar.copy(den1[:], ot_t[D:D + 1, :])
            rec_f = rpool.tile([1, NCHUNK], F32, tag="rec_f")
            nc.vector.reciprocal_approx_fast(rec_f[:], den1[:])
            bc_s = opool.tile([D, NCHUNK], F32, tag="bc")
            if h == HPC - 1 and nch == NCH - 1:
                # tail-only: PE K=1 broadcast matmul is ~2us faster
                # than the DRAM bounce, and at the very end the PE is
                # idle and HAM re-throttling no longer matters
                rec_r = rpool.tile([1, NCHUNK], F32R, tag="rec_r")
                nc.vector.tensor_copy(rec_r[:], rec_f[:])
                bc_t = s_ps.tile([D, NCHUNK], F32, tag="s",
                                 name="bc_ps")
                nc.tensor.matmul(bc_t[:], ones_r[:], rec_r[:],
                                 start=True, stop=True)
                nc.vector.tensor_copy(bc_s[:], bc_t[:])
            else:
                rec_d = rdram.tile([1, NCHUNK], F32, tag="rec_d")
                nc.sync.dma_start(rec_d[:], rec_f[:])
                nc.sync.dma_start(bc_s[:],
                                  rec_d[:].partition_broadcast(D))
            o_t = opool.tile([D, NCHUNK], F32, tag="o")
            nc.vector.tensor_mul(o_t[:], ot_t[0:D, :], bc_s[:])
            nc.sync.dma_start(ot_d[h][:, bass.ts(nch, NCHUNK)], o_t[:])

        def emit_s_exp(h, nch, pair):
            qt_c = qt_all[h][nch // 2][:, nch % 2, :]
            kp = kt_all[h][pair // PPIECE][:, pair % PPIECE, :]
            s_t = s_ps.tile([128, 2, NCHUNK], F32, tag="s")
            # concurrent K=64 row tiles: even m-block from
            # partitions 0-63, odd from 64-127
            nc.tensor.matmul(
                s_t[:, 0, :], kp[0:64, :], qt_c[0:64, :],
                start=True, stop=True, tile_position=(0, 0),
            )
            nc.tensor.matmul(
                s_t[:, 1, :], kp[64:128, :], qt_c[64:128, :],
                start=True, stop=True, tile_position=(64, 0),
            )
            if pair in DVE_PAIRS:
                p_i = pexp.tile([128, 2, NCHUNK], I16, tag="pi")
                nc.vector.tensor_scalar(
                    p_i[:], s_t[:], SCHRAU_A, SCHRAU_B,
                    mybir.AluOpType.mult, mybir.AluOpType.add,
                )
                return p_i[:].bitcast(BF16)
            p_t = pexp.tile([128, 2, NCHUNK], BF16, tag="pf")
            nc.scalar.activation(
                p_t[:], s_t[:],
                mybir.ActivationFunctionType.Exp,
                bias=bias_t[:], scale=1.0,
            )
            return p_t[:]

        # Emission in blocks of 2 same-head pairs: the PE pays ~100ns per
        # tile-geometry switch between (64,128) S-tiles and (128,128)
        # O-tiles (measured: the second V' load of a group hides in the
        # background weight buffer, the cross-geometry ones don't).
        # Batching 2 S-pairs then 4 O-matmuls halves the switch count.
        slots = [(c, p) for c in range(NCH) for p in range(NPAIR)]
        h_blocks = [slots[i:i + 2] for i in range(0, len(slots), 2)]
        order_blocks = []
        j = -(NPAIR // 4)
        for i in range(len(h_blocks)):
            order_blocks.append((0, h_blocks[i]))
            if j >= 0:
                order_blocks.append((1, h_blocks[j]))
            j += 1
        while j < len(h_blocks):
            if j >= 0:
                order_blocks.append((1, h_blocks[j]))
            j += 1

        pend = []
        for h, blk in order_blocks:
            for c, p in blk:
                p_mm = emit_s_exp(h, c, p)
                pend.append((h, c, p, p_mm))
            while len(pend) > LAG:
                emit_o(pend.pop(0))
        while pend:
            emit_o(pend.pop(0))

    nc.compile()
    return nc


def _get_nc():
    if "nc" not in _CACHE:
        _CACHE["nc"] = _build()
    return _CACHE["nc"]


def _bf16_bits(x32):
    # round-to-nearest-even fp32 -> bf16, returned as uint16 bit payload
    u = x32.astype(np.float32).view(np.uint32)
    rounded = (u + 0x7FFF + ((u >> 16) & 1)) >> 16
    return rounded.astype(np.uint16)


def _make_in_maps(QKV):
    QKV = np.asarray(QKV, dtype=np.float32)
    q = QKV[0].reshape(HEADS, N, D)
    k = QKV[1].reshape(HEADS, N, D)
    v = QKV[2].reshape(HEADS, N, D)
    qt = q.transpose(0, 2, 1)                       # [16, 64, 4096]
    # duplicate Q^T into both 64-partition halves for the row tiles
    qt_dup = np.concatenate([qt, qt], axis=1).astype(np.float16)
    kt = k.transpose(0, 2, 1).reshape(HEADS, D, NPAIR, 2, 128)
    # even m-blocks -> partitions 0-63, odd -> 64-127, pair-major cols
    kt_packed = np.concatenate([kt[:, :, :, 0], kt[:, :, :, 1]],
                               axis=1).reshape(HEADS, 128, NPAIR * 128)
    kt_packed = kt_packed.astype(np.float16)
    v_bf = _bf16_bits(v)
    in_maps = []
    for c in range(N_CORES):
        sl = slice(c * HPC, (c + 1) * HPC)
        in_maps.append({
            "qt": np.ascontiguousarray(qt_dup[sl]),
            "kt": np.ascontiguousarray(kt_packed[sl]),
            "v": np.ascontiguousarray(v_bf[sl]),
        })
    return in_maps


def _assemble(results):
    ot = np.stack([r["ot"] for r in results])            # [8, 2, 64, 4096]
    out = ot.reshape(HEADS, D, N).transpose(0, 2, 1)     # [16, 4096, 64]
    return np.ascontiguousarray(out).reshape(B, H, N, D).astype(np.float32)


def kernel(QKV):
    nc = _get_nc()
    res = run_bass_kernel_spmd(nc, _make_in_maps(QKV), list(range(N_CORES)))
    return _assemble(res.results)
